# revision 7
# baseline (speedup 1.0000x reference)
"""Trainium2 Bass kernel for CTRLightGCN-style GNN message passing block.

Reference computation (per full input):
    A_g = row_normalized(A.sum(0)) + A_group                    # (4,25,25)
    xg = x.reshape(B, 4, 64, T, V)
    y  = einsum('gdc,gvw,bgctw->bgdtv', conv_w, A_g, xg).reshape(B, C, T, V)
    out = x + BN_train(y) * gamma + beta        (BN stats over B,T,V per C)

Strategy: data-parallel over batch B=64 across 8 cores (8 per core).
Per core, per (b, channel-half) the two contractions run as a PE matmul
chain that needs no explicit transpose (fp16 inputs, fp32 PSUM accum):

  MM1:  lhsT = x16 chunk (gc=128 x 128 cols)    [x is the *stationary* op]
        rhs  = Wblk (gc=128 x gd=128, block-diag conv_w per group pair)
        out  = y1T chunk ((t,w) x gd) in PSUM     -> conv done, transposed
  MM2:  lhsT = y1T (SBUF fp16 copy) group column slice ((t,w) x 64)
        rhs  = kron(I_5, A_g^T) ((t,w) x (t,v))
        out  = y chunk (gd x (t,v)) in PSUM       -> spatial agg, natural

y stays resident in SBUF as fp16.  bn_stats/bn_aggr accumulate per-channel
partials over 500-col batches; a tiny (2,128,2) AllReduce combines
sum/sumsq across the 8 cores; pass 2 re-streams fp32 x and writes
out = x + ghat * y + delta.  A warmup burst of dummy matmuls flips the
PE HAM clock-gate to full rate before real work.
"""
import numpy as np

import concourse.bacc as bacc
import concourse.tile as tile
from concourse import mybir
from concourse.bass_utils import run_bass_kernel_spmd

# ---- problem constants (hardcoded per contract) ----
B, C, T, V = 64, 256, 128, 25
G = 4
N_CORES = 8
B_LOC = B // N_CORES          # 8
TW = T * V                    # 3200
TW_PAD = 3328                 # x16 padded so every 128-col lhsT read is in-bounds
BN_EPS = 1e-5
N_PER_CH = B * TW             # 204800 (global per-channel count)

# chunk = 5 t-rows = 125 cols (last chunk 3 t = 75); batches of 4 chunks -> <=500 cols
CHUNK_M = [125] * 25 + [75]
BATCHES = []                  # list of (f0, [m...]) per (b,h)
_f = 0
_i = 0
while _i < len(CHUNK_M):
    ms = CHUNK_M[_i:_i + 4]
    if sum(ms) > 500:
        ms = CHUNK_M[_i:_i + 2]
    BATCHES.append((_f, ms))
    _f += sum(ms)
    _i += len(ms)
N_BAT = len(BATCHES)          # 7 (6x500 + 1x200)
N_REC = B_LOC * N_BAT         # 56 bn_stats records per half

F32 = mybir.dt.float32
F16 = mybir.dt.float16

_cache = {}


def _build():
    nc = bacc.Bacc()
    x16_in = nc.dram_tensor("x16", [B_LOC, 2, 128, TW_PAD], F16, kind="ExternalInput")
    wblk_in = nc.dram_tensor("wblk", [2, 128, 128], F16, kind="ExternalInput")
    arhs_in = nc.dram_tensor("arhs", [G, 125, 125], F16, kind="ExternalInput")
    gbn_in = nc.dram_tensor("gbn", [2, 128, 2], F32, kind="ExternalInput")
    out_d = nc.dram_tensor("out", [B_LOC, C, TW], F32, kind="ExternalOutput")

    with tile.TileContext(nc) as tc:
        with (
            tc.tile_pool(name="consts", bufs=1) as consts,
            tc.tile_pool(name="resid", bufs=1) as resid,
            tc.tile_pool(name="xp", bufs=3) as xp,
            tc.tile_pool(name="op", bufs=4) as op,
            tc.tile_pool(name="y1t", bufs=3) as y1tp,
            tc.tile_pool(name="ps1", bufs=3, space="PSUM") as ps1,
            tc.tile_pool(name="ps2", bufs=3, space="PSUM") as ps2,
            tc.tile_pool(name="psw", bufs=1, space="PSUM") as psw,
            tc.tile_pool(name="dr", bufs=1, space="DRAM") as dr,
        ):
            # ---- PE HAM warmup: ~4.5us of dense dummy matmuls ----
            wtile = consts.tile([128, 128], F16, tag="warm")
            nc.vector.memset(wtile, 0.0)
            wp = psw.tile([128, 128], F32, tag="warmp")
            for _ in range(80):
                nc.tensor.matmul(wp, wtile, wtile, start=True, stop=True)
            wsink = consts.tile([128, 1], F32, tag="wsink")
            nc.scalar.copy(out=wsink, in_=wp[:, 0:1])

            # ---- constants ----
            wblk_t = []
            gbn_t = []
            arhs_t = []
            for h in range(2):
                w = consts.tile([128, 128], F16, tag=f"wblk{h}")
                nc.sync.dma_start(out=w, in_=wblk_in[h])
                wblk_t.append(w)
                gbt = consts.tile([128, 2], F32, tag=f"gbn{h}")
                nc.sync.dma_start(out=gbt, in_=gbn_in[h])
                gbn_t.append(gbt)
            for g in range(G):
                a = consts.tile([125, 125], F16, tag=f"arhs{g}")
                nc.sync.dma_start(out=a, in_=arhs_in[g])
                arhs_t.append(a)

            y16 = [resid.tile([128, B_LOC, TW], F16, tag=f"y16_{h}", name=f"y16_{h}")
                   for h in range(2)]
            statsbuf = [
                consts.tile([128, N_REC, 6], F32, tag=f"stats{h}", name=f"stats{h}")
                for h in range(2)
            ]

            # ---- pass 1 ----
            for b in range(B_LOC):
                for h in range(2):
                    xt = xp.tile([128, TW_PAD], F16, tag="xt")
                    nc.sync.dma_start(out=xt, in_=x16_in[b, h])
                    for bi, (f0, ms) in enumerate(BATCHES):
                        used = sum(ms)
                        p1 = ps1.tile([128, 4, 128], F32, tag="p1")
                        co = f0
                        for ci, m in enumerate(ms):
                            nc.tensor.matmul(
                                p1[:, ci, :], xt[:, co:co + 128], wblk_t[h],
                                start=True, stop=True,
                            )
                            co += m
                        nch = len(ms)
                        y1t = y1tp.tile([128, 4, 128], F16, tag="y1t")
                        nc.scalar.copy(
                            out=y1t[:, :nch, :], in_=p1[:, :nch, :]
                        )
                        p2 = ps2.tile([128, 500], F32, tag="p2")
                        co = 0
                        for ci, m in enumerate(ms):
                            with tc.tile_critical():
                                nc.tensor.ldweights(
                                    y1t[0:m, ci, :], tile_position=(0, 0)
                                )
                                mis = [
                                    nc.tensor.matmul(
                                        p2[gl * 64:(gl + 1) * 64, co:co + m],
                                        y1t[0:m, ci, gl * 64:(gl + 1) * 64],
                                        arhs_t[2 * h + gl][:m, :m],
                                        start=True, stop=True,
                                        tile_position=(0, gl * 64),
                                    )
                                    for gl in range(2)
                                ]
                            for mi in mis:
                                mi.ins.ldweights = False
                            co += m
                        # cast PSUM->fp16 slab (alternate engines), stats from slab
                        yslice = y16[h][:, b, f0:f0 + used]
                        if bi % 2 == 0:
                            nc.scalar.copy(out=yslice, in_=p2[:, :used])
                        else:
                            nc.vector.tensor_copy(out=yslice, in_=p2[:, :used])
                        nc.vector.bn_stats(
                            out=statsbuf[h][:, b * N_BAT + bi, :], in_=yslice
                        )

            # ---- stats: aggregate -> allreduce -> ghat/delta ----
            cc_in = dr.tile([128, 4], F32)
            cc_out = dr.tile([128, 4], F32, addr_space="Shared")
            n_loc = float(B_LOC * TW)
            sums = consts.tile([128, 4], F32, tag="sums")
            for h in range(2):
                mv = consts.tile([128, 2], F32, tag=f"mv{h}")
                nc.vector.bn_aggr(out=mv, in_=statsbuf[h])
                m2 = consts.tile([128, 1], F32, tag=f"m2{h}")
                nc.vector.tensor_mul(m2, mv[:, 0:1], mv[:, 0:1])
                nc.vector.tensor_add(m2, m2, mv[:, 1:2])
                nc.scalar.mul(out=sums[:, 2 * h + 1:2 * h + 2], in_=m2, mul=n_loc)
                nc.scalar.mul(out=sums[:, 2 * h:2 * h + 1], in_=mv[:, 0:1], mul=n_loc)
            nc.sync.dma_start(out=cc_in, in_=sums)
            nc.gpsimd.collective_compute(
                "AllReduce",
                mybir.AluOpType.add,
                replica_groups=[list(range(N_CORES))],
                ins=[cc_in[:, :]],
                outs=[cc_out[:, :]],
            )
            eps_t = consts.tile([128, 1], F32, tag="eps")
            nc.vector.memset(eps_t, BN_EPS)
            gs_all = consts.tile([128, 4], F32, tag="gs_all")
            nc.sync.dma_start(out=gs_all, in_=cc_out)
            ghat_t = []
            delta_t = []
            for h in range(2):
                gs = gs_all[:, 2 * h:2 * h + 2]
                gmean = consts.tile([128, 1], F32, tag=f"gmean{h}")
                var = consts.tile([128, 1], F32, tag=f"var{h}")
                tmp = consts.tile([128, 1], F32, tag=f"tmp{h}")
                nc.scalar.mul(out=gmean, in_=gs[:, 0:1], mul=1.0 / N_PER_CH)
                nc.scalar.mul(out=var, in_=gs[:, 1:2], mul=1.0 / N_PER_CH)
                nc.vector.tensor_mul(tmp, gmean, gmean)
                nc.vector.tensor_sub(var, var, tmp)
                nc.scalar.activation(
                    out=var, in_=var, func=mybir.ActivationFunctionType.Sqrt,
                    bias=eps_t, scale=1.0,
                )
                nc.vector.reciprocal(out=var, in_=var)
                gh = consts.tile([128, 1], F32, tag=f"ghat{h}")
                dl = consts.tile([128, 1], F32, tag=f"delta{h}")
                nc.vector.tensor_mul(gh, gbn_t[h][:, 0:1], var)
                nc.vector.tensor_mul(tmp, gmean, gh)
                nc.vector.tensor_sub(dl, gbn_t[h][:, 1:2], tmp)
                ghat_t.append(gh)
                delta_t.append(dl)

            # ---- pass 2: out = x + ghat*y16 + delta ----
            HT = TW // 2
            for b in range(B_LOC):
                for h in range(2):
                    xt2 = xp.tile([128, TW], F16, tag="xt2")
                    nc.sync.dma_start(out=xt2, in_=x16_in[b, h, :, :TW])
                    for s in range(2):
                        ot = op.tile([128, HT], F32, tag="ot")
                        nc.vector.tensor_scalar(
                            out=ot, in0=y16[h][:, b, s * HT:(s + 1) * HT],
                            scalar1=ghat_t[h], scalar2=delta_t[h],
                            op0=mybir.AluOpType.mult, op1=mybir.AluOpType.add,
                        )
                        nc.vector.tensor_add(ot, ot, xt2[:, s * HT:(s + 1) * HT])
                        nc.sync.dma_start(
                            out=out_d[b, h * 128:(h + 1) * 128, s * HT:(s + 1) * HT],
                            in_=ot,
                        )

    nc.finalize()
    return nc


def _prep_consts(A, A_group, conv_w, gamma, beta):
    A_sum = A.sum(axis=0)
    row_sum = np.clip(A_sum.sum(axis=-1, keepdims=True), 1e-6, None)
    A_g = (A_sum / row_sum)[None, :, :] + A_group          # (4,25,25)
    wblk = np.zeros((2, 128, 128), np.float16)
    for h in range(2):
        for gl in range(2):
            g = 2 * h + gl
            wblk[h, gl * 64:(gl + 1) * 64, gl * 64:(gl + 1) * 64] = \
                conv_w[g].T.astype(np.float16)
    eye = np.eye(5, dtype=np.float32)
    arhs = np.stack([np.kron(eye, A_g[g].T) for g in range(G)]).astype(np.float16)
    gbn = np.stack(
        [np.stack([gamma.reshape(2, 128)[h], beta.reshape(2, 128)[h]], axis=1)
         for h in range(2)]
    ).astype(np.float32)
    return wblk, np.ascontiguousarray(arhs), np.ascontiguousarray(gbn)


def _run(inputs, trace=False, **kw):
    if "nc" not in _cache:
        _cache["nc"] = _build()
    nc = _cache["nc"]
    x = np.asarray(inputs["x"], dtype=np.float32)
    wblk, arhs, gbn = _prep_consts(
        np.asarray(inputs["A"], np.float32),
        np.asarray(inputs["A_group"], np.float32),
        np.asarray(inputs["conv_w"], np.float32),
        np.asarray(inputs["gamma"], np.float32),
        np.asarray(inputs["beta"], np.float32),
    )
    xs = x.reshape(N_CORES, B_LOC, 2, 128, TW)
    x16 = np.zeros((N_CORES, B_LOC, 2, 128, TW_PAD), np.float16)
    x16[..., :TW] = xs.astype(np.float16)
    in_maps = [
        {"x16": np.ascontiguousarray(x16[i]), "wblk": wblk, "arhs": arhs, "gbn": gbn}
        for i in range(N_CORES)
    ]
    res = run_bass_kernel_spmd(nc, in_maps, list(range(N_CORES)), trace=trace, **kw)
    out = np.concatenate([res.results[i]["out"][None] for i in range(N_CORES)])
    return out.reshape(B, C, T, V), res


def kernel(**inputs) -> np.ndarray:
    out, _ = _run(inputs)
    return out


# revision 8
# speedup vs baseline: 3.1503x; 3.1503x over previous
"""Trainium2 Bass kernel for CTRLightGCN-style GNN message passing block.

Reference computation (per full input):
    A_g = row_normalized(A.sum(0)) + A_group                    # (4,25,25)
    xg = x.reshape(B, 4, 64, T, V)
    y  = einsum('gdc,gvw,bgctw->bgdtv', conv_w, A_g, xg).reshape(B, C, T, V)
    out = x + BN_train(y) * gamma + beta        (BN stats over B,T,V per C)

Strategy: data-parallel over batch B=64 across 8 cores (8 per core).
Per core, per (b, channel-half) the two contractions run as a PE matmul
chain that needs no explicit transpose (fp16 inputs, fp32 PSUM accum):

  MM1:  lhsT = x16 chunk (gc=128 x 128 cols)    [x is the *stationary* op]
        rhs  = Wblk (gc=128 x gd=128, block-diag conv_w per group pair)
        out  = y1T chunk ((t,w) x gd) in PSUM     -> conv done, transposed
  MM2:  lhsT = y1T (SBUF fp16 copy) group column slice ((t,w) x 64)
        rhs  = kron(I_5, A_g^T) ((t,w) x (t,v))
        out  = y chunk (gd x (t,v)) in PSUM       -> spatial agg, natural

y stays resident in SBUF as fp16.  bn_stats/bn_aggr accumulate per-channel
partials over 500-col batches; a tiny (2,128,2) AllReduce combines
sum/sumsq across the 8 cores; pass 2 re-streams fp32 x and writes
out = x + ghat * y + delta.  A warmup burst of dummy matmuls flips the
PE HAM clock-gate to full rate before real work.
"""
import numpy as np

import concourse.bacc as bacc
import concourse.tile as tile
from concourse import mybir
from concourse.bass_utils import run_bass_kernel_spmd

# ---- problem constants (hardcoded per contract) ----
B, C, T, V = 64, 256, 128, 25
G = 4
N_CORES = 8
B_LOC = B // N_CORES          # 8
TW = T * V                    # 3200
TW_PAD = 3328                 # x16 padded so every 128-col lhsT read is in-bounds
BN_EPS = 1e-5
N_PER_CH = B * TW             # 204800 (global per-channel count)

# chunk = 5 t-rows = 125 cols (last chunk 3 t = 75); batches of 4 chunks -> <=500 cols
CHUNK_M = [125] * 25 + [75]
BATCHES = []                  # list of (f0, [m...]) per (b,h)
_f = 0
_i = 0
while _i < len(CHUNK_M):
    ms = CHUNK_M[_i:_i + 4]
    if sum(ms) > 500:
        ms = CHUNK_M[_i:_i + 2]
    BATCHES.append((_f, ms))
    _f += sum(ms)
    _i += len(ms)
N_BAT = len(BATCHES)          # 7 (6x500 + 1x200)
N_REC = B_LOC * N_BAT         # 56 bn_stats records per half

F32 = mybir.dt.float32
F16 = mybir.dt.float16

_cache = {}


def _build():
    nc = bacc.Bacc()
    x16_in = nc.dram_tensor("x16", [B_LOC, 2, 128, TW_PAD], F16, kind="ExternalInput")
    wblk_in = nc.dram_tensor("wblk", [2, 128, 128], F16, kind="ExternalInput")
    arhs_in = nc.dram_tensor("arhs", [G, 125, 125], F16, kind="ExternalInput")
    gbn_in = nc.dram_tensor("gbn", [2, 128, 2], F32, kind="ExternalInput")
    out_d = nc.dram_tensor("out", [B_LOC, C, TW], F32, kind="ExternalOutput")

    with tile.TileContext(nc) as tc:
        with (
            tc.tile_pool(name="consts", bufs=1) as consts,
            tc.tile_pool(name="resid", bufs=1) as resid,
            tc.tile_pool(name="xp", bufs=3) as xp,
            tc.tile_pool(name="op", bufs=4) as op,
            tc.tile_pool(name="y1t", bufs=3) as y1tp,
            tc.tile_pool(name="ps1", bufs=3, space="PSUM") as ps1,
            tc.tile_pool(name="ps2", bufs=3, space="PSUM") as ps2,
            tc.tile_pool(name="psw", bufs=1, space="PSUM") as psw,
            tc.tile_pool(name="dr", bufs=1, space="DRAM") as dr,
        ):
            # ---- PE HAM warmup: ~4.5us of dense dummy matmuls ----
            wtile = consts.tile([128, 128], F16, tag="warm")
            nc.vector.memset(wtile, 0.0)
            wp = psw.tile([128, 128], F32, tag="warmp")
            for _ in range(80):
                nc.tensor.matmul(wp, wtile, wtile, start=True, stop=True)
            wsink = consts.tile([128, 1], F32, tag="wsink")
            nc.scalar.copy(out=wsink, in_=wp[:, 0:1])

            # ---- constants ----
            wblk_t = []
            gbn_t = []
            arhs_t = []
            for h in range(2):
                w = consts.tile([128, 128], F16, tag=f"wblk{h}")
                nc.sync.dma_start(out=w, in_=wblk_in[h])
                wblk_t.append(w)
                gbt = consts.tile([128, 2], F32, tag=f"gbn{h}")
                nc.sync.dma_start(out=gbt, in_=gbn_in[h])
                gbn_t.append(gbt)
            for g in range(G):
                a = consts.tile([125, 125], F16, tag=f"arhs{g}")
                nc.sync.dma_start(out=a, in_=arhs_in[g])
                arhs_t.append(a)

            y16 = [resid.tile([128, B_LOC, TW], F16, tag=f"y16_{h}", name=f"y16_{h}")
                   for h in range(2)]
            statsbuf = [
                consts.tile([128, N_REC, 6], F32, tag=f"stats{h}", name=f"stats{h}")
                for h in range(2)
            ]

            # ---- pass 1 ----
            for b in range(B_LOC):
                for h in range(2):
                    xt = xp.tile([128, TW_PAD], F16, tag="xt")
                    nc.sync.dma_start(out=xt, in_=x16_in[b, h])
                    for bi, (f0, ms) in enumerate(BATCHES):
                        used = sum(ms)
                        p1 = ps1.tile([128, 4, 128], F32, tag="p1")
                        co = f0
                        for ci, m in enumerate(ms):
                            nc.tensor.matmul(
                                p1[:, ci, :], xt[:, co:co + 128], wblk_t[h],
                                start=True, stop=True,
                            )
                            co += m
                        nch = len(ms)
                        y1t = y1tp.tile([128, 4, 128], F16, tag="y1t")
                        nc.scalar.copy(
                            out=y1t[:, :nch, :], in_=p1[:, :nch, :]
                        )
                        p2 = ps2.tile([128, 500], F32, tag="p2")
                        co = 0
                        for ci, m in enumerate(ms):
                            for gl in range(2):
                                nc.tensor.matmul(
                                    p2[gl * 64:(gl + 1) * 64, co:co + m],
                                    y1t[0:m, ci, gl * 64:(gl + 1) * 64],
                                    arhs_t[2 * h + gl][:m, :m],
                                    start=True, stop=True,
                                    tile_position=(0, gl * 64),
                                )
                            co += m
                        # cast PSUM->fp16 slab (alternate engines), stats from slab
                        yslice = y16[h][:, b, f0:f0 + used]
                        if bi % 2 == 0:
                            nc.scalar.copy(out=yslice, in_=p2[:, :used])
                        else:
                            nc.vector.tensor_copy(out=yslice, in_=p2[:, :used])
                        nc.vector.bn_stats(
                            out=statsbuf[h][:, b * N_BAT + bi, :], in_=yslice
                        )

            # ---- stats: aggregate -> allreduce -> ghat/delta ----
            cc_in = dr.tile([128, 4], F32)
            cc_out = dr.tile([128, 4], F32, addr_space="Shared")
            n_loc = float(B_LOC * TW)
            sums = consts.tile([128, 4], F32, tag="sums")
            for h in range(2):
                mv = consts.tile([128, 2], F32, tag=f"mv{h}")
                nc.vector.bn_aggr(out=mv, in_=statsbuf[h])
                m2 = consts.tile([128, 1], F32, tag=f"m2{h}")
                nc.vector.tensor_mul(m2, mv[:, 0:1], mv[:, 0:1])
                nc.vector.tensor_add(m2, m2, mv[:, 1:2])
                nc.scalar.mul(out=sums[:, 2 * h + 1:2 * h + 2], in_=m2, mul=n_loc)
                nc.scalar.mul(out=sums[:, 2 * h:2 * h + 1], in_=mv[:, 0:1], mul=n_loc)
            nc.sync.dma_start(out=cc_in, in_=sums)
            nc.gpsimd.collective_compute(
                "AllReduce",
                mybir.AluOpType.add,
                replica_groups=[list(range(N_CORES))],
                ins=[cc_in[:, :]],
                outs=[cc_out[:, :]],
            )
            eps_t = consts.tile([128, 1], F32, tag="eps")
            nc.vector.memset(eps_t, BN_EPS)
            gs_all = consts.tile([128, 4], F32, tag="gs_all")
            nc.sync.dma_start(out=gs_all, in_=cc_out)
            ghat_t = []
            delta_t = []
            for h in range(2):
                gs = gs_all[:, 2 * h:2 * h + 2]
                gmean = consts.tile([128, 1], F32, tag=f"gmean{h}")
                var = consts.tile([128, 1], F32, tag=f"var{h}")
                tmp = consts.tile([128, 1], F32, tag=f"tmp{h}")
                nc.scalar.mul(out=gmean, in_=gs[:, 0:1], mul=1.0 / N_PER_CH)
                nc.scalar.mul(out=var, in_=gs[:, 1:2], mul=1.0 / N_PER_CH)
                nc.vector.tensor_mul(tmp, gmean, gmean)
                nc.vector.tensor_sub(var, var, tmp)
                nc.scalar.activation(
                    out=var, in_=var, func=mybir.ActivationFunctionType.Sqrt,
                    bias=eps_t, scale=1.0,
                )
                nc.vector.reciprocal(out=var, in_=var)
                gh = consts.tile([128, 1], F32, tag=f"ghat{h}")
                dl = consts.tile([128, 1], F32, tag=f"delta{h}")
                nc.vector.tensor_mul(gh, gbn_t[h][:, 0:1], var)
                nc.vector.tensor_mul(tmp, gmean, gh)
                nc.vector.tensor_sub(dl, gbn_t[h][:, 1:2], tmp)
                ghat_t.append(gh)
                delta_t.append(dl)

            # ---- pass 2: out = x + ghat*y16 + delta ----
            HT = TW // 2
            for b in range(B_LOC):
                for h in range(2):
                    xt2 = xp.tile([128, TW], F16, tag="xt2")
                    nc.sync.dma_start(out=xt2, in_=x16_in[b, h, :, :TW])
                    for s in range(2):
                        ot = op.tile([128, HT], F32, tag="ot")
                        nc.vector.tensor_scalar(
                            out=ot, in0=y16[h][:, b, s * HT:(s + 1) * HT],
                            scalar1=ghat_t[h], scalar2=delta_t[h],
                            op0=mybir.AluOpType.mult, op1=mybir.AluOpType.add,
                        )
                        nc.vector.tensor_add(ot, ot, xt2[:, s * HT:(s + 1) * HT])
                        nc.sync.dma_start(
                            out=out_d[b, h * 128:(h + 1) * 128, s * HT:(s + 1) * HT],
                            in_=ot,
                        )

    nc.finalize()
    return nc


def _prep_consts(A, A_group, conv_w, gamma, beta):
    A_sum = A.sum(axis=0)
    row_sum = np.clip(A_sum.sum(axis=-1, keepdims=True), 1e-6, None)
    A_g = (A_sum / row_sum)[None, :, :] + A_group          # (4,25,25)
    wblk = np.zeros((2, 128, 128), np.float16)
    for h in range(2):
        for gl in range(2):
            g = 2 * h + gl
            wblk[h, gl * 64:(gl + 1) * 64, gl * 64:(gl + 1) * 64] = \
                conv_w[g].T.astype(np.float16)
    eye = np.eye(5, dtype=np.float32)
    arhs = np.stack([np.kron(eye, A_g[g].T) for g in range(G)]).astype(np.float16)
    gbn = np.stack(
        [np.stack([gamma.reshape(2, 128)[h], beta.reshape(2, 128)[h]], axis=1)
         for h in range(2)]
    ).astype(np.float32)
    return wblk, np.ascontiguousarray(arhs), np.ascontiguousarray(gbn)


def _run(inputs, trace=False, **kw):
    if "nc" not in _cache:
        _cache["nc"] = _build()
    nc = _cache["nc"]
    x = np.asarray(inputs["x"], dtype=np.float32)
    wblk, arhs, gbn = _prep_consts(
        np.asarray(inputs["A"], np.float32),
        np.asarray(inputs["A_group"], np.float32),
        np.asarray(inputs["conv_w"], np.float32),
        np.asarray(inputs["gamma"], np.float32),
        np.asarray(inputs["beta"], np.float32),
    )
    xs = x.reshape(N_CORES, B_LOC, 2, 128, TW)
    x16 = np.zeros((N_CORES, B_LOC, 2, 128, TW_PAD), np.float16)
    x16[..., :TW] = xs.astype(np.float16)
    in_maps = [
        {"x16": np.ascontiguousarray(x16[i]), "wblk": wblk, "arhs": arhs, "gbn": gbn}
        for i in range(N_CORES)
    ]
    res = run_bass_kernel_spmd(nc, in_maps, list(range(N_CORES)), trace=trace, **kw)
    out = np.concatenate([res.results[i]["out"][None] for i in range(N_CORES)])
    return out.reshape(B, C, T, V), res


def kernel(**inputs) -> np.ndarray:
    out, _ = _run(inputs)
    return out
